# revision 11
# baseline (speedup 1.0000x reference)
"""FLGC (fully-learnable group conv) Trainium2 kernel.

Strategy
--------
Host (numpy, cheap): routing = argmax over S/T rows (256x8), fold the
softmax scales + group gate into the 3x3 conv weights, sort channels by
group, pack groups into 64-wide input "bins" and 32-wide output
"streams", pad x to a 58x58 grid (zero border) in fp16.

Device (8 cores, data-parallel over batch, 4 images/core): the grouped
conv runs as 9 shifted matmuls per output stream per 512-column spatial
chunk, accumulated in PSUM.  Each stream is a (64 rows x 32 cols)
tile_position matmul so up to 8 independent streams run concurrently on
the 128x128 PE array.  fp16 operands, fp32 PSUM accumulate, fp32 output.

Host finally crops the padding, restores original channel order and
applies the reference's perm[perm] output permutation.
"""

import os
import sys

sys.path.insert(0, "/opt/trn_rl_repo")

import numpy as np

B, CIN, COUT, H, W, G, KH = 32, 256, 256, 56, 56, 8, 3
NCORES, BL = 8, 4           # cores, images per core
PW = 58                     # padded row width (56 + 2)
IMGLEN = 3368               # per-image x stride, >= 56*58 + 118 = 3366
NOUT = 56 * PW              # 3248 output columns per image (2 garbage cols/row)
XCOLS = BL * IMGLEN         # 13472
OCOLS = BL * NOUT           # 12992
NTAP = 9
SPCH = [(i * 512, min(512, NOUT - i * 512)) for i in range((NOUT + 511) // 512)]

LAST_EXEC_NS = None
LAST_TRACE = None

_CACHE = {}


def _softmax64(a):
    a = a.astype(np.float64)
    e = np.exp(a - a.max(axis=1, keepdims=True))
    return e / e.sum(axis=1, keepdims=True)


def _set_partitions(items):
    """All partitions of a list into unordered groups."""
    if not items:
        yield []
        return
    first, rest = items[0], items[1:]
    for p in _set_partitions(rest):
        for i in range(len(p)):
            yield p[:i] + [[first] + p[i]] + p[i + 1:]
        yield p + [[first]]


def _plan(S, T):
    """Compute routing + packing metadata (deterministic from S, T)."""
    s = np.asarray(S).argmax(axis=1)        # group per input channel (exact)
    t = np.asarray(T).argmax(axis=1)        # group per output filter (exact)
    s_hat = _softmax64(np.asarray(S))
    t_hat = _softmax64(np.asarray(T))
    s_scale = s_hat[np.arange(CIN), s]      # scale per input channel
    t_scale = t_hat[np.arange(COUT), t]     # scale per output filter

    ins = [np.where(s == g)[0] for g in range(G)]
    outs = [np.where(t == g)[0] for g in range(G)]
    ci = np.array([len(v) for v in ins])
    co = np.array([len(v) for v in outs])

    # --- pack groups into bins: sum(ci) <= 64 per bin; minimize
    # (total output streams, #bins).
    best = None
    for part in _set_partitions(list(range(G))):
        if any(ci[list(bn)].sum() > 64 for bn in part):
            continue
        streams = sum(-(-co[list(bn)].sum() // 32) for bn in part)
        key = (streams, len(part))
        if best is None or key < best[0]:
            best = (key, [sorted(bn) for bn in part])
    assert best is not None
    bins = best[1]

    # streams per bin
    bin_streams = [-(-int(co[list(bn)].sum()) // 32) for bn in bins]

    # --- assign bins to array halves (row 0-63 / 64-127), balancing streams
    order = sorted(range(len(bins)), key=lambda b: -bin_streams[b])
    half_of = {}
    loads = [0, 0]
    counts = [0, 0]
    for b in order:
        h = 0 if (loads[0], counts[0]) <= (loads[1], counts[1]) else 1
        half_of[b] = h
        loads[h] += bin_streams[b]
        counts[h] += 1
    assert max(loads) <= 5, f"streams per half {loads} > 5 unsupported"
    assert max(counts) <= 3, f"bins per half {counts} > 3 unsupported"
    nch = max(counts)

    # chunk index per bin (within its half)
    chunk_of = {}
    cidx = [0, 0]
    for b in range(len(bins)):
        h = half_of[b]
        chunk_of[b] = cidx[h]
        cidx[h] += 1

    # --- streams: per half, list of {bin, out channel ids, k, wcol}
    halves = {0: [], 1: []}
    for b, bn in enumerate(bins):
        h = half_of[b]
        och = np.concatenate([outs[g] for g in bn])
        for blk in range(0, len(och), 32):
            chans = och[blk:blk + 32]
            halves[h].append({
                "bin": b, "chunk": chunk_of[b], "half": h,
                "chans": chans, "nv": len(chans),
            })
    for h in (0, 1):
        for k, st in enumerate(halves[h]):
            st["k"] = k
            st["wcol"] = k * (NTAP * 32)

    # global output row layout
    row = 0
    chan_of_row = np.zeros(COUT, dtype=np.int64)
    for h in (0, 1):
        for st in halves[h]:
            st["row0"] = row
            chan_of_row[row:row + st["nv"]] = st["chans"]
            row += st["nv"]
    assert row == COUT

    # input channel layout per bin (padded to 64 rows)
    bin_in = []
    for bn in bins:
        bin_in.append(np.concatenate([ins[g] for g in bn]))

    wcols = max(len(halves[0]), len(halves[1])) * NTAP * 32
    perm = np.argsort(t * COUT + np.arange(COUT), kind="stable")

    return {
        "s": s, "t": t, "s_scale": s_scale, "t_scale": t_scale,
        "bins": bins, "bin_in": bin_in, "half_of": half_of,
        "chunk_of": chunk_of, "halves": halves, "nch": nch,
        "wcols": wcols, "chan_of_row": chan_of_row, "perm": perm,
    }


def _plan_key(meta):
    parts = [tuple(meta["s"].tolist()), tuple(meta["t"].tolist())]
    return hash(tuple(parts))


def _col_of(k, img, h, chunk=0):
    # base streams (k<4): static column per image; overflow stream (k>=4):
    # rotate per spatial chunk so the doubled subarray cell migrates and
    # no cell is the pacer for a whole image.
    if k < 4:
        return (k + img + 2 * h) % 4
    return (img + chunk + 2 * h) % 4


def _patch_ldw_opt():
    """Optionally enable walrus LDWEIGHTS dedup (skip reload of an identical
    stationary operand) — gated by BASS_LDW_OPT=1."""
    if os.environ.get("BASS_LDW_OPT", "0") != "1":
        return
    import concourse.bass_utils as bu
    if getattr(bu.run_command, "_ldw_patched", False):
        return
    orig = bu.run_command

    def run_command(cmd, *a, **kw):
        cmd = ["--enable-ldw-opt=true" if c == "--enable-ldw-opt=false" else c
               for c in cmd]
        return orig(cmd, *a, **kw)

    run_command._ldw_patched = True
    bu.run_command = run_command


def _build(meta):
    import concourse.bacc as bacc
    import concourse.mybir as mybir
    from concourse.tile import TileContext, add_dep_helper

    dt = mybir.dt
    nch = meta["nch"]
    halves = meta["halves"]
    use_ov = len(halves[0]) > 4 or len(halves[1]) > 4

    # used partition range per x chunk
    chunk_rows = {}
    for b, bn in enumerate(meta["bins"]):
        h = meta["half_of"][b]
        c = meta["chunk_of"][b]
        lo, hi = 64 * h, 64 * h + 64
        if c in chunk_rows:
            chunk_rows[c] = (min(chunk_rows[c][0], lo), max(chunk_rows[c][1], hi))
        else:
            chunk_rows[c] = (lo, hi)

    nc = bacc.Bacc("TRN2", target_bir_lowering=False, debug=False,
                   num_devices=NCORES)
    xin = nc.dram_tensor("xin", [128 * nch, XCOLS], dt.float16,
                         kind="ExternalInput").ap()
    win = nc.dram_tensor("win", [128, meta["wcols"]], dt.float16,
                         kind="ExternalInput").ap()
    # out rows: img-major, then [O0, O1, OV2] tiles of 128 partitions each
    oud = nc.dram_tensor("out", [BL * 3 * 128, NOUT], dt.float16,
                         kind="ExternalOutput").ap()

    with TileContext(nc) as tc:
        with tc.tile_pool(name="const", bufs=1) as cp, \
             tc.tile_pool(name="ob", bufs=6) as obp, \
             tc.tile_pool(name="ps", bufs=8, space="PSUM") as psp:
            # -- PE warmup: dummy matmuls with no DMA dependency keep the PE
            # busy through the HAM activity window while the first x slice
            # streams in, so real matmuls start at 2.4 GHz instead of 1.2.
            nwarm = int(os.environ.get("BASS_WARM_MMS", "10"))
            warm_chain = None
            if nwarm:
                warm = cp.tile([128, 512], dt.float16, tag="warm")
                nc.gpsimd.memset(warm[0:64, 0:512], 0.0)
                wps = psp.tile([128, 512], dt.float32, tag="ps", name="warmps")
                for i in range(nwarm):
                    wmm = nc.tensor.matmul(
                        wps[0:32, 0:512], warm[0:64, 0:32], warm[0:64, 0:512],
                        start=(i == 0), stop=(i == nwarm - 1),
                        tile_position=(0, 0))
                    if warm_chain is not None:
                        add_dep_helper(wmm.ins, warm_chain.ins, sync=False,
                                       reason="warmup order")
                    warm_chain = wmm
            wt = cp.tile([128, meta["wcols"]], dt.float16, tag="w")
            nc.sync.dma_start(out=wt[:, :], in_=win[:, :])
            xtile = cp.tile([128, nch * XCOLS], dt.float16, tag="x",
                            name="xtile")
            xh = IMGLEN // 2
            full = [c for c in range(nch) if chunk_rows[c] == (0, 128)]
            part = [c for c in range(nch) if chunk_rows[c] != (0, 128)]
            # contiguous run of full-height chunks for a single 3D-AP DMA
    # (chunks are laid out side by side in xtile and row-blocked in xin)
            xdst3 = xtile[0:128, :].rearrange("p (c n) -> p c n", c=nch)
    # xin rows are chunk-major blocks of 128
            xsrc3 = xin[:, :].rearrange("(c p) n -> p c n", c=nch)
            assert full == list(range(len(full)))
            nf = len(full)
            last_x_dma = None
            xdma_of_img = {img: [] for img in range(BL)}
            for img in range(BL):
                slices = ((0, xh // 2), (xh // 2, xh), (xh, IMGLEN)) \
                    if img == 0 else ((0, IMGLEN),)
                for a, bnd in slices:
                    if nf:
                        last_x_dma = nc.sync.dma_start(
                            out=xdst3[:, 0:nf,
                                      img * IMGLEN + a:img * IMGLEN + bnd],
                            in_=xsrc3[:, 0:nf,
                                      img * IMGLEN + a:img * IMGLEN + bnd])
                        xdma_of_img[img].append(last_x_dma)
                    for c in part:
                        lo, hi = chunk_rows[c]
                        last_x_dma = nc.sync.dma_start(
                            out=xtile[lo:hi, c * XCOLS + img * IMGLEN + a:
                                      c * XCOLS + img * IMGLEN + bnd],
                            in_=xin[128 * c + lo:128 * c + hi,
                                    img * IMGLEN + a:img * IMGLEN + bnd])
                        xdma_of_img[img].append(last_x_dma)

            prev_mm = warm_chain
            first_mm_of_img = {}
            anchor_mm_of_img = {}
            deferred_outs = []
            for img in range(BL):
                O0 = obp.tile([128, NOUT], dt.float16, tag="ob")
                O1 = obp.tile([128, NOUT], dt.float16, tag="ob")
                OV = obp.tile([128, 4 * 512], dt.float16, tag="ov", name="OV") \
                    if use_ov else None
                Otab = {0: O0, 1: O1}

                # pair-rounds: two spatial chunks with identical weights per
                # cell, so LDWEIGHTS amortizes over 2 matmuls (with ldw-opt).
                # sub-round task: (h, k, chunk, col)
                pair_rounds = []
                for pr in range(0, len(SPCH), 2):
                    subs = []
                    for ch in (pr, pr + 1):
                        if ch >= len(SPCH):
                            continue
                        tasks = []
                        for h in (0, 1):
                            for k, st in enumerate(halves[h][:4]):
                                col = (k + img + 2 * h) % 4
                                tasks.append((h, k, ch, col))
                        subs.append(("main", ch, tasks))
                    pair_rounds.append(subs)
                if use_ov:
                    subs = []
                    for f in range(2):
                        tasks = []
                        for h in (0, 1):
                            if len(halves[h]) <= 4:
                                continue
                            for c in range(4 * f, min(4 * f + 4, len(SPCH))):
                                col = (c + img + 2 * h) % 4
                                tasks.append((h, 4, c, col))
                        subs.append(("fl", f, tasks))
                    pair_rounds.insert(1, subs)

                eng_flip = 0
                for ridx, subs in enumerate(pair_rounds):
                    # allocate one psum bank per (sub-round, half)
                    P = {}
                    for si, (kind, rid, tasks) in enumerate(subs):
                        for h in (0, 1):
                            P[(si, h)] = psp.tile([128, 512], dt.float32,
                                                  tag="ps", name=f"P{si}{h}")
                    for tap in range(NTAP):
                        dh, dw = divmod(tap, 3)
                        toff = dh * PW + dw
                        # cell-major, sub-round inner: consecutive matmuls on
                        # one cell share the stationary operand
                        cells = {}
                        for si, (kind, rid, tasks) in enumerate(subs):
                            for (h, k, ch, col) in tasks:
                                cells.setdefault((col, h), []).append(
                                    (si, h, k, ch, col))
                        for cell in sorted(cells):
                            for (si, h, k, ch, col) in cells[cell]:
                                st = halves[h][k]
                                c0, ncols = SPCH[ch]
                                xb = img * IMGLEN + c0 + toff
                                mm = nc.tensor.matmul(
                                    P[(si, h)][32 * col:32 * col + 32, 0:ncols],
                                    wt[64 * h:64 * h + 64,
                                       st["wcol"] + tap * 32:
                                       st["wcol"] + tap * 32 + 32],
                                    xtile[64 * h:64 * h + 64,
                                          st["chunk"] * XCOLS + xb:
                                          st["chunk"] * XCOLS + xb + ncols],
                                    start=(tap == 0), stop=(tap == NTAP - 1),
                                    tile_position=(64 * h, 32 * col),
                                )
                                if prev_mm is not None and \
                                        os.environ.get("BASS_MM_CHAIN", "1") == "1":
                                    add_dep_helper(mm.ins, prev_mm.ins,
                                                   sync=False,
                                                   reason="pe issue order")
                                prev_mm = mm
                                if img not in first_mm_of_img:
                                    first_mm_of_img[img] = mm
                                if ridx == 1 and img not in anchor_mm_of_img:
                                    anchor_mm_of_img[img] = mm
                    # evacuate: every copy is a full 128-partition bank copy,
                    # split across both engines by column half
                    for si, (kind, rid, tasks) in enumerate(subs):
                        for h in (0, 1):
                            if not any(t[0] == h for t in tasks):
                                continue
                            if kind == "main":
                                c0, ncols = SPCH[rid]
                                dst = Otab[h]
                                d0 = c0
                            else:
                                ncols = 512
                                dst = OV
                                d0 = (2 * h + rid) * 512
                            if (eng_flip + h) % 2 == 0:
                                nc.vector.tensor_copy(
                                    dst[:, d0:d0 + ncols],
                                    P[(si, h)][:, 0:ncols])
                            else:
                                nc.scalar.copy(
                                    dst[:, d0:d0 + ncols],
                                    P[(si, h)][:, 0:ncols])
                    eng_flip += 1

                    # drain this round's output columns to HBM now
                    if subs[0][0] == "main":
                        a = SPCH[subs[0][1]][0]
                        lc0, lnc = SPCH[subs[-1][1]]
                        for ti, tile_ in ((0, O0), (1, O1)):
                            r0 = img * 384 + ti * 128
                            od = nc.gpsimd.dma_start(
                                out=oud[r0:r0 + 128, a:lc0 + lnc],
                                in_=tile_[0:128, a:lc0 + lnc])
                            deferred_outs.append((img, od))
                    else:
                        r0 = img * 384 + 2 * 128
                        od = nc.gpsimd.dma_start(
                            out=oud[r0:r0 + 128, 0:4 * 512],
                            in_=OV[0:128, 0:4 * 512])
                        deferred_outs.append((img, od))
            for img_o, od in deferred_outs:
                add_dep_helper(od.ins, last_x_dma.ins, sync=False,
                               reason="x before out")
                gate = first_mm_of_img.get(min(img_o + 1, BL - 1))
                if gate is not None and img_o + 1 <= BL - 1:
                    add_dep_helper(od.ins, gate.ins, sync=True,
                                   reason="defer outs vs ifetch")
            # pace late images' x input: hold img>=2 x transfers until the
            # img-2 compute is underway so early instruction-page fetches and
            # the first images' x don't fight a saturated DMA fabric.
            for img in range(2, BL):
                gate = anchor_mm_of_img.get(img - 2)
                if gate is None:
                    continue
                for xd in xdma_of_img[img]:
                    add_dep_helper(xd.ins, gate.ins, sync=True,
                                   reason="pace x input")
    nc.compile()
    _dedup_ldweights(nc)
    if os.environ.get("BASS_THIN_SEMS", "1") == "1":
        _thin_mm_sems(nc)
    return nc


def _dedup_ldweights(nc):
    """Remove back-to-back identical weight loads per PE tile position.

    Pair-rounds issue two matmuls with the same stationary operand on each
    subarray cell; tile legalization still emits one Ldweights per matmul.
    The second load is redundant — drop it when it carries no sync."""
    import concourse.mybir as mybir
    removed = 0
    for blk in nc.m.functions[0].blocks:
        insts = blk.instructions
        last = {}
        drop = []
        for idx, i in enumerate(insts):
            if isinstance(i, mybir.InstMatmult):
                continue
            if not isinstance(i, mybir.InstLdweights):
                continue
            ap = i.ins[0]
            sig = (ap.memref, ap.offset, str(ap.ap), i.is_transpose)
            pos = tuple(i.tile_position or (0, 0))
            si = i.sync_info
            clean = si is None or (not si.on_wait and not si.on_update)
            if last.get(pos) == sig and clean:
                drop.append(idx)
                removed += 1
            else:
                last[pos] = sig
        for idx in reversed(drop):
            del insts[idx]
    if removed:
        import logging
        logging.getLogger(__name__).info(f"ldweights dedup: removed {removed}")


def _thin_mm_sems(nc):
    """Drop the per-matmul semaphore increment from matmuls whose completion
    nobody observes.

    Tile gives every InstMatmult a `sem++@complete` on the PE counting
    semaphore; the EVT_SEM register write serializes at ~26ns per increment,
    which at 2520 matmuls adds up to tens of microseconds of PE issue stall.
    Matmul completions are in pc order, so a wait on count>=k is a wait on
    the k-th matmul completing.  Keep increments only on (a) accumulation-
    group finals (stop=True) and (b) the exact k-th matmul targeted by any
    wait, then renumber every wait threshold in the reduced counting.  The
    targeted matmul of each wait is unchanged, so the dependency graph (and
    deadlock-freedom) is preserved."""
    import concourse.mybir as mybir

    blocks = nc.m.functions[0].blocks
    sem_votes = {}
    mms = []
    for blk in blocks:
        for i in blk.instructions:
            if isinstance(i, mybir.InstMatmult):
                mms.append(i)
                for u in (i.sync_info.on_update if i.sync_info else []):
                    sem_votes[u.id] = sem_votes.get(u.id, 0) + 1
    if not sem_votes:
        return
    sem = max(sem_votes, key=sem_votes.get)
    if sem_votes[sem] < len(mms):
        return  # unexpected shape; leave untouched

    for i in mms:
        for u in i.sync_info.on_update:
            if u.id == sem:
                assert u.update_mode == "sem-inc" and u.update_value == 1, u

    keep = [bool(i.stop_tensor_calc) for i in mms]
    waits = []  # (instr, wait) pairs referencing `sem`
    for blk in blocks:
        for i in blk.instructions:
            si = getattr(i, "sync_info", None)
            if si is None:
                continue
            for w in (si.on_wait or []):
                if w.id == sem:
                    assert w.wait_mode == "sem-ge-imm" and w.wait_reg is None, w
                    assert 1 <= w.wait_value <= len(mms), w
                    keep[w.wait_value - 1] = True
                    waits.append(w)

    # prefix sums of kept increments
    pref = [0]
    for k in keep:
        pref.append(pref[-1] + (1 if k else 0))
    for w in waits:
        w.wait_value = pref[w.wait_value]

    removed = 0
    for i, k in zip(mms, keep):
        if not k:
            i.sync_info.on_update = [
                u for u in i.sync_info.on_update if u.id != sem]
            removed += 1
    import logging
    logging.getLogger(__name__).info(
        f"mm sem thinning: kept {len(mms) - removed}/{len(mms)} increments")


def _prep_inputs(x, conv, meta):
    # fold scales + gate into weights
    s, t = meta["s"], meta["t"]
    wfull = (np.asarray(conv).astype(np.float64)
             * meta["t_scale"][:, None, None, None]
             * meta["s_scale"][None, :, None, None]
             * (s[None, :] == t[:, None])[:, :, None, None].astype(np.float64))
    wfull = wfull.astype(np.float32)

    wt = np.zeros((128, meta["wcols"]), dtype=np.float16)
    for h in (0, 1):
        for st in meta["halves"][h]:
            bch = meta["bin_in"][st["bin"]]          # input channels of bin
            blk = wfull[np.ix_(st["chans"], bch)]    # [m, kk, 3, 3]
            # lhsT[kk, m] per tap
            for tap in range(NTAP):
                dh, dw = divmod(tap, 3)
                wt[64 * h:64 * h + len(bch),
                   st["wcol"] + tap * 32:st["wcol"] + tap * 32 + st["nv"]] = \
                    blk[:, :, dh, dw].T.astype(np.float16)

    # padded fp16 x, channels re-ordered into bins
    xpad = np.zeros((B, CIN, 58, 58), dtype=np.float16)
    xpad[:, :, 1:57, 1:57] = np.asarray(x)
    flat = xpad.reshape(B, CIN, 58 * 58)
    nch = meta["nch"]
    xg = np.zeros((128 * nch, B, IMGLEN), dtype=np.float16)
    for b, bn in enumerate(meta["bins"]):
        h = meta["half_of"][b]
        c = meta["chunk_of"][b]
        bch = meta["bin_in"][b]
        r0 = 128 * c + 64 * h
        xg[r0:r0 + len(bch), :, :58 * 58] = flat[:, bch].transpose(1, 0, 2)

    in_maps = []
    for core in range(NCORES):
        xc = np.ascontiguousarray(
            xg[:, BL * core:BL * core + BL].reshape(128 * nch, XCOLS))
        in_maps.append({"xin": xc, "win": wt})
    return in_maps


def _install_ntff_hook_shim():
    """Provide antenv.axon_hooks (absent in this image) so
    run_bass_kernel_spmd(trace=True) can NTFF-profile via the axon .so."""
    import sys as _sys
    if "antenv.axon_hooks" in _sys.modules:
        return
    import contextlib
    import ctypes
    import types

    so_path = "/opt/axon/libaxon_pjrt.so"
    hook = None
    try:
        lib = ctypes.CDLL(so_path)
        if hasattr(lib, "axon_start_nrt_profile"):
            lib.axon_start_nrt_profile.argtypes = [
                ctypes.POINTER(ctypes.c_int64), ctypes.c_size_t]
            lib.axon_start_nrt_profile.restype = ctypes.c_int64
            lib.axon_stop_nrt_profile.argtypes = [ctypes.c_char_p]
            lib.axon_stop_nrt_profile.restype = ctypes.c_int64

            @contextlib.contextmanager
            def _hook(output_dir, device_ids):
                import jax
                jax.devices()
                if device_ids:
                    ids = (ctypes.c_int64 * len(device_ids))(*device_ids)
                    rc = lib.axon_start_nrt_profile(ids, len(device_ids))
                else:
                    rc = lib.axon_start_nrt_profile(None, 0)
                if rc != 0:
                    raise RuntimeError(f"axon_start_nrt_profile rc={rc}")
                try:
                    yield
                finally:
                    n = lib.axon_stop_nrt_profile(str(output_dir).encode())
                    if n < 0:
                        raise RuntimeError(f"axon_stop_nrt_profile rc={n}")
                    print(f"profile: {n} file(s) written to {output_dir}",
                          file=sys.stderr)

            hook = _hook
    except OSError:
        pass

    mod = types.ModuleType("antenv.axon_hooks")
    mod.get_axon_ntff_profile_hook = lambda: hook
    mod.set_axon_ntff_profile_hook = lambda h: None
    _sys.modules["antenv.axon_hooks"] = mod


def kernel(x, conv, S, T):
    global LAST_EXEC_NS, LAST_TRACE
    from concourse.bass_utils import run_bass_kernel_spmd

    meta = _plan(S, T)
    key = _plan_key(meta)
    if key not in _CACHE:
        _CACHE[key] = _build(meta)
    nc = _CACHE[key]

    in_maps = _prep_inputs(x, conv, meta)
    _patch_ldw_opt()
    trace = os.environ.get("BASS_PROBLEM_TRACE", "0") == "1"
    if trace:
        _install_ntff_hook_shim()
    res = run_bass_kernel_spmd(nc, in_maps, list(range(NCORES)), trace=trace)
    LAST_EXEC_NS = res.exec_time_ns
    if res.instructions_and_trace is not None:
        LAST_TRACE = res.instructions_and_trace[1]

    # assemble: gather stream rows, crop pad cols, restore channel order,
    # apply perm[perm]
    out_full = np.empty((B, COUT, 56, 56), dtype=np.float32)
    for core in range(NCORES):
        dev = res.results[core]["out"]          # [BL*384, NOUT]
        for img in range(BL):
            base = img * 384
            for h in (0, 1):
                for k, st in enumerate(meta["halves"][h]):
                    nv = st["nv"]
                    if k < 4:
                        col = (k + img + 2 * h) % 4
                        r0 = base + h * 128 + 32 * col
                        data = dev[r0:r0 + nv]
                    else:
                        data = np.empty((nv, NOUT), np.float32)
                        for c, (c0, ncols) in enumerate(SPCH):
                            col = (c + img + 2 * h) % 4
                            f = c // 4
                            r0 = base + 2 * 128 + 32 * col
                            s0 = (2 * h + f) * 512
                            data[:, c0:c0 + ncols] = \
                                dev[r0:r0 + nv, s0:s0 + ncols]
                    arr = data.reshape(nv, 56, PW)[:, :, :56]
                    out_full[BL * core + img, st["chans"]] = \
                        arr.astype(np.float32)
    pp = meta["perm"][meta["perm"]]
    return out_full[:, pp]

